# revision 19
# baseline (speedup 1.0000x reference)
"""Trainium2 Bass kernel for MultiHeadAttention (B=2, S=2048, D=1024, H=16).

Sharding: 8 cores = 2 batches x 4 head-groups (4 heads each).
Per core (batch b, 4 heads):
  - Q/K projections in transposed layout (lhsT = W blocks, rhs = actT input),
    V projection in natural layout (lhsT = vT blocks, rhs = Wv) -- no on-device
    transposes anywhere.
  - causal softmax(QK^T/8) materialized in [q,k] orientation, contiguous DMA of
    the causal region only (runtime zero-fills the rest of the output buffer).
  - A@V via recomputed [k,q]-oriented exp(S^T); a ones-column appended to each
    head's vh slice makes PSUM row 64 the softmax denominator; Y normalized in
    PSUM via reciprocal + gpsimd partition-broadcast.
  - per-head output projection (K=64 blocks); host sums the 4 head-group
    partials per batch and adds bo.
"""

import sys

import numpy as np

sys.path.insert(0, "/opt/trn_rl_repo")

B, S, D, H = 2, 2048, 1024, 16
DH = 64
P = 128
NCORES = 8
CPB = 4
HPC = H // CPB        # 4 heads per core
PAIRS = HPC // 2
WCOLS = HPC * DH      # 256
VW = DH + 1           # 65: head slice width in vh (data + ones column)
NEG = -1.0e9
SCALE = 1.0 / 8.0

_prog_cache: dict = {}
_last_results = None


def _build_program(S_=S, D_=D, debug_dump=False, use_bias=True):
    import concourse.mybir as mybir
    from concourse import bacc, tile

    f32 = mybir.dt.float32
    Exp = mybir.ActivationFunctionType.Exp
    AX = mybir.AxisListType.X
    ADD = mybir.AluOpType.add

    SB = S_ // P
    DB = D_ // P
    NT = S_ // 512
    ND = D_ // 512

    nc = bacc.Bacc(None, target_bir_lowering=False, debug=False)

    qT_d = nc.dram_tensor("qT", [D_, S_], f32, kind="ExternalInput")
    kT_d = nc.dram_tensor("kT", [D_, S_], f32, kind="ExternalInput")
    vT_d = nc.dram_tensor("vT", [D_, S_], f32, kind="ExternalInput")
    Wq_d = nc.dram_tensor("Wq", [D_, WCOLS], f32, kind="ExternalInput")
    Wk_d = nc.dram_tensor("Wk", [D_, WCOLS], f32, kind="ExternalInput")
    Wv_d = nc.dram_tensor("Wv", [D_, WCOLS], f32, kind="ExternalInput")
    Wo_d = nc.dram_tensor("Wo", [DH, HPC, D_], f32, kind="ExternalInput")
    bq_d = nc.dram_tensor("bq", [1, WCOLS], f32, kind="ExternalInput")
    bk_d = nc.dram_tensor("bk", [1, WCOLS], f32, kind="ExternalInput")
    bv_d = nc.dram_tensor("bv", [1, WCOLS], f32, kind="ExternalInput")
    negB_d = nc.dram_tensor("negB", [P, P], f32, kind="ExternalInput")
    negC_d = nc.dram_tensor("negC", [P, 896], f32, kind="ExternalInput")
    attn_d = nc.dram_tensor("attn_o", [HPC, S_, S_], f32, kind="ExternalOutput")
    y_d = nc.dram_tensor("y_o", [S_, D_], f32, kind="ExternalOutput")
    if debug_dump:
        dbg = {
            "qhT_dump": [P, PAIRS, S_], "khT_dump": [P, PAIRS, S_],
            "vh_dump": [P, SB, HPC * VW], "Yc_dump": [P, HPC, S_],
            "rcol_dump": [P, HPC, SB],
        }
        dbg_d = {k2: nc.dram_tensor(k2, shp, f32, kind="ExternalOutput")
                 for k2, shp in dbg.items()}

    with tile.TileContext(nc) as tc:
        with tc.tile_pool(name="persist", bufs=1) as pers:
            # ---- constants ----
            Wo_sb = pers.tile([P, HPC, D_], f32, tag="Wo", name="Wo")
            nc.sync.dma_start(Wo_sb[0:DH, :, :], Wo_d[:, :, :])
            bq_sb = pers.tile([1, WCOLS], f32, tag="bq", name="bq")
            nc.sync.dma_start(bq_sb[:], bq_d[:, :])
            bk_sb = pers.tile([1, WCOLS], f32, tag="bk", name="bk")
            nc.sync.dma_start(bk_sb[:], bk_d[:, :])
            bv_sb = pers.tile([1, WCOLS], f32, tag="bv", name="bv")
            nc.sync.dma_start(bv_sb[:], bv_d[:, :])
            negB_sb = pers.tile([P, P], f32, tag="negB", name="negB")
            nc.sync.dma_start(negB_sb[:], negB_d[:, :])
            negC_sb = pers.tile([P, 896], f32, tag="negC", name="negC")
            nc.sync.dma_start(negC_sb[:], negC_d[:, :])
            ones_sb = pers.tile([1, 512], f32, tag="ones", name="ones")
            nc.vector.memset(ones_sb[:], 1.0)

            # ---- long-lived activations ----
            qhT_sb = pers.tile([P, PAIRS, S_], f32, tag="qhT", name="qhT")
            khT_sb = pers.tile([P, PAIRS, S_], f32, tag="khT", name="khT")
            # natural v-heads + ones col: vh_sb[p, sb, h*65 + j]
            vh_sb = pers.tile([P, SB, HPC * VW], f32, tag="vh", name="vh")
            nc.vector.memset(vh_sb[:], 1.0)
            # normalized Y^T per head (partitions 0:64)
            Yc_sb = pers.tile([P, HPC, S_], f32, tag="Yc", name="Yc")
            # attn softmax reciprocal sums, q on partitions
            rcol_sb = pers.tile([P, HPC, SB], f32, tag="rcol", name="rcol")

            # ---- phase 1a: q/k projections (transposed layout) ----
            for actT_dram, W_dram, b_sb, outT in (
                (qT_d, Wq_d, bq_sb, qhT_sb),
                (kT_d, Wk_d, bk_sb, khT_sb),
            ):
                with tc.tile_pool(name="projpsum", bufs=2 * NT, space="PSUM") as pp, \
                     tc.tile_pool(name="actin", bufs=2) as ain, \
                     tc.tile_pool(name="wp", bufs=1) as wp:
                    W_sb = wp.tile([P, DB, WCOLS], f32, tag="W", name="W")
                    nc.sync.dma_start(
                        W_sb[:], W_dram[:, :].rearrange("(o p) m -> p o m", p=P))
                    psums = [pp.tile([P, 512], f32, tag="pp", name="pp")
                             for _ in range(PAIRS * NT)]
                    for db in range(DB):
                        blk = ain.tile([P, S_], f32, tag="blk", name="blk")
                        nc.sync.dma_start(blk[:], actT_dram[db * P:(db + 1) * P, :])
                        for m in range(PAIRS):
                            for nt in range(NT):
                                nc.tensor.matmul(
                                    psums[m * NT + nt][:],
                                    lhsT=W_sb[:, db, m * P:(m + 1) * P],
                                    rhs=blk[:, nt * 512:(nt + 1) * 512],
                                    start=(db == 0),
                                    stop=(not use_bias and db == DB - 1))
                    for m in range(PAIRS):
                        for nt in range(NT):
                            if use_bias:
                                nc.tensor.matmul(
                                    psums[m * NT + nt][:],
                                    lhsT=b_sb[0:1, m * P:(m + 1) * P],
                                    rhs=ones_sb[0:1, :],
                                    start=False, stop=True)
                            nc.any.tensor_copy(
                                out=outT[:, m, nt * 512:(nt + 1) * 512],
                                in_=psums[m * NT + nt][:])

            # ---- phase 1b: v projection, natural layout, two streamed passes ----
            HSB = SB // 2
            with tc.tile_pool(name="vpp", bufs=HSB, space="PSUM") as vpp, \
                 tc.tile_pool(name="vactin", bufs=2) as vain, \
                 tc.tile_pool(name="vwp", bufs=1) as vwp:
                Wv_sb = vwp.tile([P, DB, WCOLS], f32, tag="Wv", name="Wv")
                nc.sync.dma_start(
                    Wv_sb[:], Wv_d[:, :].rearrange("(o p) m -> p o m", p=P))
                for half in range(2):
                    vps = [vpp.tile([P, WCOLS], f32, tag="vp", name="vp")
                           for _ in range(HSB)]
                    for db in range(DB):
                        blk = vain.tile([P, S_ // 2], f32, tag="vblk", name="vblk")
                        nc.sync.dma_start(
                            blk[:],
                            vT_d[db * P:(db + 1) * P,
                                 half * (S_ // 2):(half + 1) * (S_ // 2)])
                        for sl in range(HSB):
                            nc.tensor.matmul(
                                vps[sl][:],
                                lhsT=blk[:, sl * P:(sl + 1) * P],
                                rhs=Wv_sb[:, db, :],
                                start=(db == 0),
                                stop=(not use_bias and db == DB - 1))
                    for sl in range(HSB):
                        sb = half * HSB + sl
                        if use_bias:
                            nc.tensor.matmul(
                                vps[sl][:],
                                lhsT=ones_sb[0:1, 0:P],
                                rhs=bv_sb[0:1, :],
                                start=False, stop=True)
                        for h in range(HPC):
                            nc.any.tensor_copy(
                                out=vh_sb[:, sb, h * VW:h * VW + DH],
                                in_=vps[sl][:, h * DH:(h + 1) * DH])

            # ---- phase 2: attention per head-pair ----
            with tc.tile_pool(name="sp", bufs=4, space="PSUM") as sp, \
                 tc.tile_pool(name="yp", bufs=2, space="PSUM") as yp, \
                 tc.tile_pool(name="ep", bufs=2) as ep, \
                 tc.tile_pool(name="etp", bufs=4) as etp, \
                 tc.tile_pool(name="smp", bufs=8) as smp:
                for pr in range(PAIRS):
                    h0, h1 = 2 * pr, 2 * pr + 1
                    # --- B: attn output strips ([q,k] orientation) ---
                    for sb in range(SB):
                        kl = (sb + 1) * P
                        nch = (kl + 511) // 512
                        E0 = ep.tile([P, S_], f32, tag="E0", name="E0")
                        E1 = ep.tile([P, S_], f32, tag="E1", name="E1")
                        su40 = smp.tile([P, 4], f32, tag="su40", name="su40")
                        su41 = smp.tile([P, 4], f32, tag="su41", name="su41")
                        for c in range(nch):
                            w = min(512, kl - c * 512)
                            ps0 = sp.tile([P, 512], f32, tag="sp", name="sp")
                            ps1 = sp.tile([P, 512], f32, tag="sp", name="sp")
                            nc.tensor.matmul(
                                ps0[:, :w],
                                lhsT=qhT_sb[0:64, pr, sb * P:(sb + 1) * P],
                                rhs=khT_sb[0:64, pr, c * 512:c * 512 + w],
                                start=True, stop=True)
                            nc.tensor.matmul(
                                ps1[:, :w],
                                lhsT=qhT_sb[64:128, pr, sb * P:(sb + 1) * P],
                                rhs=khT_sb[64:128, pr, c * 512:c * 512 + w],
                                start=True, stop=True)
                            if c == nch - 1:
                                nc.vector.tensor_add(
                                    out=ps0[:, w - P:w], in0=ps0[:, w - P:w],
                                    in1=negB_sb[:])
                                nc.vector.tensor_add(
                                    out=ps1[:, w - P:w], in0=ps1[:, w - P:w],
                                    in1=negB_sb[:])
                            nc.scalar.activation(
                                E0[:, c * 512:c * 512 + w], ps0[:, :w], Exp,
                                scale=SCALE, accum_out=su40[:, c:c + 1])
                            nc.scalar.activation(
                                E1[:, c * 512:c * 512 + w], ps1[:, :w], Exp,
                                scale=SCALE, accum_out=su41[:, c:c + 1])
                        su0 = smp.tile([P, 1], f32, tag="su0", name="su0")
                        su1 = smp.tile([P, 1], f32, tag="su1", name="su1")
                        nc.vector.tensor_reduce(su0[:], su40[:, 0:nch], AX, ADD)
                        nc.vector.tensor_reduce(su1[:], su41[:, 0:nch], AX, ADD)
                        nc.vector.reciprocal(rcol_sb[:, h0, sb:sb + 1], su0[:])
                        nc.vector.reciprocal(rcol_sb[:, h1, sb:sb + 1], su1[:])
                        nc.vector.tensor_scalar_mul(
                            E0[:, 0:kl], E0[:, 0:kl], rcol_sb[:, h0, sb:sb + 1])
                        nc.vector.tensor_scalar_mul(
                            E1[:, 0:kl], E1[:, 0:kl], rcol_sb[:, h1, sb:sb + 1])
                        nc.sync.dma_start(
                            attn_d[h0, sb * P:(sb + 1) * P, 0:kl], E0[:, 0:kl])
                        nc.sync.dma_start(
                            attn_d[h1, sb * P:(sb + 1) * P, 0:kl], E1[:, 0:kl])
                    # --- C: A@V via S^T orientation; su in PSUM row 64 ---
                    for qt in range(NT):
                        Yps0 = yp.tile([P, 512], f32, tag="Y0", name="Yps0")
                        Yps1 = yp.tile([P, 512], f32, tag="Y1", name="Yps1")
                        nkb = 4 * qt + 4
                        for kb in range(nkb):
                            ps0 = sp.tile([P, 512], f32, tag="sp", name="sp")
                            ps1 = sp.tile([P, 512], f32, tag="sp", name="sp")
                            nc.tensor.matmul(
                                ps0[:],
                                lhsT=khT_sb[0:64, pr, kb * P:(kb + 1) * P],
                                rhs=qhT_sb[0:64, pr, qt * 512:(qt + 1) * 512],
                                start=True, stop=True)
                            nc.tensor.matmul(
                                ps1[:],
                                lhsT=khT_sb[64:128, pr, kb * P:(kb + 1) * P],
                                rhs=qhT_sb[64:128, pr, qt * 512:(qt + 1) * 512],
                                start=True, stop=True)
                            o = kb - 4 * qt
                            if o >= 0:
                                nsl = negC_sb[:, 384 - 128 * o: 384 - 128 * o + 512]
                                nc.vector.tensor_add(out=ps0[:], in0=ps0[:], in1=nsl)
                                nc.vector.tensor_add(out=ps1[:], in0=ps1[:], in1=nsl)
                            Et0 = etp.tile([P, 512], f32, tag="Et", name="Et")
                            Et1 = etp.tile([P, 512], f32, tag="Et", name="Et")
                            nc.scalar.activation(Et0[:], ps0[:], Exp, scale=SCALE)
                            nc.scalar.activation(Et1[:], ps1[:], Exp, scale=SCALE)
                            nc.tensor.matmul(
                                Yps0[0:VW, :],
                                lhsT=vh_sb[:, kb, h0 * VW:(h0 + 1) * VW],
                                rhs=Et0[:],
                                start=(kb == 0), stop=(kb == nkb - 1))
                            nc.tensor.matmul(
                                Yps1[0:VW, :],
                                lhsT=vh_sb[:, kb, h1 * VW:(h1 + 1) * VW],
                                rhs=Et1[:],
                                start=(kb == 0), stop=(kb == nkb - 1))
                        for Yps, hh in ((Yps0, h0), (Yps1, h1)):
                            nc.any.tensor_copy(
                                out=Yc_sb[0:64, hh, qt * 512:(qt + 1) * 512],
                                in_=Yps[0:64, :])

            # ---- phase 3: output projection, per head (K=64); the per-head
            # softmax denominator is applied here as a per-partition ACT scale
            # on each head's partial, then partials are summed on DVE ----
            Cp = mybir.ActivationFunctionType.Copy
            with tc.tile_pool(name="op", bufs=2, space="PSUM") as op, \
                 tc.tile_pool(name="yo", bufs=2) as yo, \
                 tc.tile_pool(name="yh", bufs=2) as yhp:
                for sb in range(SB):
                    yt = yo.tile([P, D_], f32, tag="yt", name="yt")
                    for nt in range(ND):
                        nsl = slice(nt * 512, (nt + 1) * 512)
                        for h in range(HPC):
                            yps = op.tile([P, 512], f32, tag=f"opp{h}",
                                          name=f"opp{h}")
                            nc.tensor.matmul(
                                yps[:],
                                lhsT=Yc_sb[0:64, h, sb * P:(sb + 1) * P],
                                rhs=Wo_sb[0:DH, h, nt * 512:(nt + 1) * 512],
                                start=True, stop=True)
                            if h == 0:
                                nc.scalar.activation(
                                    yt[:, nsl], yps[:], Cp,
                                    scale=rcol_sb[:, h, sb:sb + 1])
                            else:
                                yh = yhp.tile([P, 512], f32, tag="yh", name="yh")
                                nc.scalar.activation(
                                    yh[:], yps[:], Cp,
                                    scale=rcol_sb[:, h, sb:sb + 1])
                                nc.vector.tensor_add(
                                    out=yt[:, nsl], in0=yt[:, nsl], in1=yh[:])
                    nc.sync.dma_start(y_d[sb * P:(sb + 1) * P, :], yt[:])

            if debug_dump:
                for k2, t2 in (("qhT_dump", qhT_sb), ("khT_dump", khT_sb),
                               ("vh_dump", vh_sb), ("Yc_dump", Yc_sb),
                               ("rcol_dump", rcol_sb)):
                    nc.sync.dma_start(dbg_d[k2][:], t2[:])

    nc.compile()
    return nc


def _host_consts():
    i = np.arange(P)
    negB = np.where(i[None, :] > i[:, None], NEG, 0.0).astype(np.float32)
    negC = np.where(np.arange(896)[None, :] < i[:, None] + 384, NEG, 0.0).astype(
        np.float32)
    return negB, negC


def _numpy_fallback(q, k, v, mask, Wq, bq, Wk, bk, Wv, bv, Wo, bo):
    def split_heads(x):
        return x.reshape(B, S, H, DH).transpose(0, 2, 1, 3)

    qh = split_heads(q @ Wq + bq)
    kh = split_heads(k @ Wk + bk)
    vh = split_heads(v @ Wv + bv)
    scores = np.einsum("bhqd,bhkd->bhqk", qh, kh) / np.float32(np.sqrt(DH))
    m = mask[:, None, :, :]
    scores = np.where(m == 0, np.float32(NEG), scores).astype(np.float32)
    scores -= scores.max(axis=-1, keepdims=True)
    e = np.exp(scores)
    attn = (e / e.sum(axis=-1, keepdims=True)).astype(np.float32)
    attn = np.where(m == 0, np.float32(0), attn)
    y = np.einsum("bhqk,bhkd->bhqd", attn, vh)
    y = y.transpose(0, 2, 1, 3).reshape(B, S, D) @ Wo + bo
    return y.astype(np.float32), attn


def kernel(q, k, v, mask, Wq, bq, Wk, bk, Wv, bv, Wo, bo):
    q = np.ascontiguousarray(np.asarray(q, np.float32))
    k = np.ascontiguousarray(np.asarray(k, np.float32))
    v = np.ascontiguousarray(np.asarray(v, np.float32))
    mask = np.asarray(mask, np.float32)
    Wq, Wk, Wv, Wo = (np.ascontiguousarray(np.asarray(w, np.float32))
                      for w in (Wq, Wk, Wv, Wo))
    bq, bk, bv, bo = (np.asarray(b_, np.float32) for b_ in (bq, bk, bv, bo))

    tril = np.tril(np.ones((S, S), np.float32))
    if mask.shape != (B, S, S) or not all(
            np.array_equal(mask[b_], tril) for b_ in range(B)):
        return _numpy_fallback(q, k, v, mask, Wq, bq, Wk, bk, Wv, bv, Wo, bo)

    use_bias = bool(np.any(bq) or np.any(bk) or np.any(bv))
    key = f"nc_bias{int(use_bias)}"
    if key not in _prog_cache:
        _prog_cache[key] = _build_program(use_bias=use_bias)
    nc = _prog_cache[key]

    negB, negC = _host_consts()
    in_maps = []
    for c in range(NCORES):
        b_, hg = c // CPB, c % CPB
        cs = slice(hg * WCOLS, (hg + 1) * WCOLS)
        # Wo rows for this head-group, reshaped to [64, heads, D]
        Wo_slice = np.ascontiguousarray(
            Wo[cs, :].reshape(HPC, DH, D).transpose(1, 0, 2))
        in_maps.append(dict(
            qT=np.ascontiguousarray(q[b_].T),
            kT=np.ascontiguousarray(k[b_].T),
            vT=np.ascontiguousarray(v[b_].T),
            Wq=np.ascontiguousarray(Wq[:, cs]),
            Wk=np.ascontiguousarray(Wk[:, cs]),
            Wv=np.ascontiguousarray(Wv[:, cs]),
            Wo=Wo_slice,
            bq=np.ascontiguousarray(bq[cs])[None, :],
            bk=np.ascontiguousarray(bk[cs])[None, :],
            bv=np.ascontiguousarray(bv[cs])[None, :],
            negB=negB, negC=negC,
        ))

    from concourse.bass_utils import run_bass_kernel_spmd
    bkr = run_bass_kernel_spmd(nc, in_maps, list(range(NCORES)))
    global _last_results
    _last_results = bkr
    res = bkr.results

    y = np.zeros((B, S, D), np.float32)
    attn = np.empty((B, H, S, S), np.float32)
    for c in range(NCORES):
        b_, hg = c // CPB, c % CPB
        attn[b_, hg * HPC:(hg + 1) * HPC] = res[c]["attn_o"]
        y[b_] += res[c]["y_o"]
    y += bo[None, None, :]
    return y, attn
